# revision 14
# baseline (speedup 1.0000x reference)
"""Trainium2 Bass kernel for nn_CNNGenerator (frame CNN + FC + window-sum + FC).

Key algebraic facts exploited (validated vs the reference):
  * softmax over a size-1 axis == 1.0, so the whole attention_conv stack is
    dead code; the bmm reduces to an 8-wide sliding-window sum of ffc.
  * The per-window stride-2 conv stack collapses into global conv streams:
    an "interior" stream g{1,2,3} and a "left-edge" stream e{1,2,3} per
    layer, plus a 2-tap combine (z).  Per t:
      g1[s] = b1 + sum_k W1k x[s+k-8]          e1[t] = b1 + W11 x[t-7] + W12 x[t-6]
      g2[s] = b2 + V0 G1[s-2] + V1 G1[s] + V2 G1[s+2]
      e2[t] = b2 + V1 E1[t] + V2 G1[t+2]
      g3[s] = b3 + U0 G2[s-4] + U1 G2[s] + U2 G2[s+4]
      e3[t] = b3 + U1 E2[t] + U2 G2[t+4]
      z[t]  = b4 + T1 E3[t] + T2 G3[t+8]
    (capitals = leaky-activated streams), then fc1/fc2/fc3+tanh,
    ws[t] = sum_{d=-3..4} ffc[t+d], out = clip(fcw @ ws, 0, 1).

Sharding: pure data parallel, 2 batch elements per core on 8 cores.
On-chip layout: time axis split in 4 chunks of 2048; 32-channel streams pack
4 chunks x 32ch on the 128 partitions, 64-channel streams pack 2 chunks x 64ch
(two tiles).  Weights are host-packed into block-diagonal lhsT matrices.
This revision adds: input pre-chunked per time-chunk on the host (contiguous
HBM blocks), input/weight DMAs split across the SP and Activation hardware
DGE queues so compute starts earlier, weight pack loaded in two stages
(layer-1 slots first), window-sum tree chunked to 1024-col pieces so the
final fc matmuls pipeline with it, and output DMA split in column halves
for earlier drain.
"""
import sys

if '/opt/trn_rl_repo' not in sys.path:
    sys.path.insert(0, '/opt/trn_rl_repo')

import numpy as np
import ml_dtypes

BF16 = ml_dtypes.bfloat16

B, C, T = 16, 29, 8192
NCORES = 8
BPC = B // NCORES          # batch per core
Tc = T // 4                # time chunk
HL = 16
W = Tc + 40                # per-batch stream tile width
W2 = BPC * W
NSLOT = 21                 # 128-col lhsT slots in the weight pack
NWCOL = NSLOT * 128 + 8    # + bias columns

_PROG = {}
PS_GROUP = 1024
PS_BUFS = 2


def _blockdiag(blocks):
    k = sum(b.shape[0] for b in blocks)
    m = sum(b.shape[1] for b in blocks)
    out = np.zeros((k, m), np.float32)
    i = j = 0
    for b in blocks:
        out[i:i + b.shape[0], j:j + b.shape[1]] = b
        i += b.shape[0]
        j += b.shape[1]
    return out


def _pack_weights(inp):
    wp = np.zeros((128, NWCOL), np.float32)

    def put(slot, mat):
        wp[:mat.shape[0], slot * 128: slot * 128 + mat.shape[1]] = mat

    w1 = inp['w1'].astype(np.float32)  # [32, 29, 3]
    b1 = inp['b1'].astype(np.float32)
    for k in range(3):
        blk = np.zeros((30, 32), np.float32)
        blk[:29] = w1[:, :, k].T
        if k == 1:
            blk[29] = b1
        put(k, _blockdiag([blk] * 4))
    w2 = inp['w2'].astype(np.float32)
    for k in range(3):  # slots 3-5
        put(3 + k, _blockdiag([w2[:, :, k].T.astype(np.float32)] * 4))
    w3 = inp['w3'].astype(np.float32)
    for k in range(3):  # slots 6-8 (chunk-pair 0) / 17-19 (pair 1): zero-padded
        # to 128 contraction rows so the matmul runs in full 128x128 mode
        blk = _blockdiag([w3[:, :, k].T.astype(np.float32)] * 2)
        zb = np.zeros_like(blk)
        put(6 + k, np.concatenate([blk, zb], axis=0))
        put(17 + k, np.concatenate([zb, blk], axis=0))
    w4 = inp['w4'].astype(np.float32)
    for j in (1, 2):    # slots 9-10
        put(9 + j - 1, _blockdiag([w4[:, :, j].T.astype(np.float32)] * 2))
    fw1t = inp['fw1'].T.astype(np.float32)
    zf = np.zeros_like(fw1t)
    put(11, np.concatenate([fw1t, zf], axis=0))
    put(20, np.concatenate([zf, fw1t], axis=0))
    fw2t = inp['fw2'].T.astype(np.float32)
    z64 = np.zeros_like(fw2t)
    put(12, np.concatenate([fw2t, z64], axis=1))
    put(15, np.concatenate([z64, fw2t], axis=1))
    fw3t = _blockdiag([inp['fw3'].T.astype(np.float32)] * 2)
    z64b = np.zeros_like(fw3t)
    put(13, np.concatenate([fw3t, z64b], axis=1))
    put(16, np.concatenate([z64b, fw3t], axis=1))
    fcb = _blockdiag([inp['fcw'].T.astype(np.float32)] * 4)  # [128, 64]
    put(14, np.concatenate([fcb, np.zeros_like(fcb)], axis=1))  # M=128
    bc = NSLOT * 128
    wp[:, bc + 0] = np.tile(inp['b2'], 4)
    wp[:, bc + 1] = np.tile(inp['b3'], 2)
    wp[:, bc + 2] = np.tile(inp['b4'], 2)
    wp[:, bc + 3] = inp['fb1']
    wp[:, bc + 4] = np.tile(inp['fb2'], 2)
    wp[:, bc + 5] = np.tile(inp['fb3'], 4)
    return wp


def _split(lo, hi, step=512):
    return [(a, min(a + step, hi)) for a in range(lo, hi, step)]



def _build_program(reps=1):
    import concourse.bacc as bacc
    import concourse.mybir as mybir
    import concourse.tile as tile

    F32 = mybir.dt.float32
    F32R = mybir.dt.float32r
    BF = mybir.dt.bfloat16
    AF = mybir.ActivationFunctionType
    OP = mybir.AluOpType

    nc = bacc.Bacc("TRN2", target_bir_lowering=False, debug=False)
    x_d = nc.dram_tensor("x", [BPC, 4, C + 1, Tc + 22], BF, kind="ExternalInput").ap()
    w1_d = nc.dram_tensor("wpack1", [120, 384], BF, kind="ExternalInput").ap()
    w_d = nc.dram_tensor("wpack", [128, NWCOL], BF, kind="ExternalInput").ap()
    o_d = nc.dram_tensor("out", [BPC, 4, 16, Tc], BF, kind="ExternalOutput").ap()

    with tile.TileContext(nc) as tc:
        with tc.tile_pool(name="wp", bufs=1) as wpool, \
             tc.tile_pool(name="xp", bufs=1) as xpool, \
             tc.tile_pool(name="yp", bufs=1) as ypool, \
             tc.tile_pool(name="st", bufs=10) as spool, \
             tc.tile_pool(name="lk", bufs=2) as lkpool, \
             tc.tile_pool(name="ps", bufs=PS_BUFS, space="PSUM") as ppool:

            wsb = wpool.tile([128, NWCOL], BF, tag="w")
            wsb1 = wpool.tile([120, 384], BF, tag="w1")
            bfs = wpool.tile([128, 8], F32, tag="bf")
            nc.scalar.dma_start(out=wsb1[:], in_=w1_d[:])

            def lhsT(slot, k=128, m=128, base=0):
                return wsb[base:base + k, slot * 128: slot * 128 + m]

            def lhsT1(slot):
                return wsb1[0:120, slot * 128: slot * 128 + 128]

            def bias(i):
                return bfs[:, i:i + 1]

            X = xpool.tile([120, W2], BF, tag="x")
            nc.sync.dma_start(out=X[0:120, 0 * W + 6: 0 * W + Tc + 28],
                              in_=x_d[0])
            nc.sync.dma_start(out=X[0:120, 1 * W + 6: 1 * W + Tc + 28],
                              in_=x_d[1])
            nc.sync.dma_start(out=wsb[:, NSLOT * 128:],
                              in_=w_d[:, NSLOT * 128:])
            nc.scalar.activation(bfs[:, 0:6],
                                 wsb[:, NSLOT * 128: NSLOT * 128 + 6],
                                 mybir.ActivationFunctionType.Copy)
            nc.sync.dma_start(out=wsb[:, 0: NSLOT * 128],
                              in_=w_d[:, 0: NSLOT * 128])

            # PE warm-up: ~3.4us of dummy matmuls during the input DMAs flips
            # the HAM clock gate to 8/8 before real work arrives.
            wmt = xpool.tile([128, 512], F32, tag="wm")
            nc.gpsimd.memset(wmt[:], 0.0)
            psw = ppool.tile([128, PS_GROUP], F32, tag="ps", name="warm")
            for _w in range(8):
                nc.tensor.matmul(psw[:, 0:512], wmt[0:128, 0:128].bitcast(F32R),
                                 wmt[:, 0:512].bitcast(F32R),
                                 start=True, stop=True)

            ST = lambda nm: spool.tile([128, W2], BF, tag="st", name=nm)  # noqa: E731

            # Round-robin: every DVE_SHAREth full conv group evacuates on DVE
            # instead of Act, spreading evac load uniformly across the
            # timeline (whole-stream assignment phase-imbalances the engines).
            evcnt = [0]
            DVE_SHARE = 4

            def conv_pass(out_tile, rng, groups, evac, bs=None, cast=True,
                          runt_evac=None, evac_dve=None):
                merge = bs is None
                for b in ([None] if merge else bs):
                    for (glo, ghi) in _split(rng[0], rng[1], PS_GROUP):
                        gn = ghi - glo
                        if gn <= 16:
                            ev = evac
                        elif evac_dve is not None:
                            ev = evac_dve if evcnt[0] % DVE_SHARE == (DVE_SHARE - 1) \
                                else evac
                            evcnt[0] += 1
                        else:
                            ev = evac
                        ps = ppool.tile([128, 2 * PS_GROUP], F32, tag="ps",
                                        name="ps")
                        for bi, bb in enumerate(range(BPC) if merge else [b]):
                            for (p0, p1, taps) in groups:
                                for (lo, hi) in _split(glo, ghi, 512):
                                    n, off = hi - lo, lo - glo
                                    for i, (lw, rt, rp0, rp1, d) in enumerate(taps):
                                        tp = (lw.base_partition(), p0) if p0 else None
                                        r = rt[rp0:rp1, bb * W + lo + d: bb * W + hi + d]
                                        nc.tensor.matmul(
                                            ps[p0:p1, bi * gn + off: bi * gn + off + n],
                                            lw, r,
                                            start=(i == 0), stop=(i == len(taps) - 1),
                                            tile_position=tp)
                        if merge:
                            # one evac instruction covers both batches via a
                            # 3D out AP (batch stride W in the stream tile)
                            pin = ps[:, 0:2 * gn].rearrange("p (b c) -> p b c", b=2)
                            ot = out_tile.rearrange("p (b c) -> p b c", b=2)[
                                :, :, glo:ghi]
                            ev(pin, ot)
                        else:
                            ev(ps[:, 0:gn], out_tile[:, b * W + glo: b * W + ghi])

            def act_evac(func, bias_ap, alpha, rnd=True):
                def f(ps, ot):
                    nc.scalar.activation(ot, ps, func, bias=bias_ap, scale=1.0,
                                         alpha=alpha)
                return f

            def dve_leaky(alpha, bias_ap=None):
                def f(ps, ot):
                    lk = lkpool.tile([128, 2 * PS_GROUP], BF, tag="lk", name="lk")
                    n = ps.free_size()
                    if bias_ap is None:
                        nc.vector.tensor_scalar(lk[:, 0:n], ps, alpha, None,
                                                OP.mult)
                        nc.vector.tensor_tensor(ot, ps, lk[:, 0:n], OP.max)
                    else:
                        nc.vector.tensor_scalar(lk[:, 0:n], ps, bias_ap, alpha,
                                                OP.add, OP.mult)
                        nc.vector.scalar_tensor_tensor(ot, ps, bias_ap,
                                                       lk[:, 0:n], OP.add, OP.max)
                return f

            def pool_leaky(alpha, bias_ap=None):
                def f(ps, ot):
                    lk = lkpool.tile([128, PS_GROUP], BF, tag="plk", name="plk")
                    n = ps.shape[-1]
                    if bias_ap is None:
                        nc.gpsimd.tensor_scalar(lk[:, 0:n], ps, alpha, None,
                                                OP.mult)
                        nc.gpsimd.tensor_tensor(ot, ps, lk[:, 0:n], OP.max)
                    else:
                        nc.gpsimd.tensor_scalar(lk[:, 0:n], ps, bias_ap, alpha,
                                                OP.add, OP.mult)
                        nc.gpsimd.scalar_tensor_tensor(ot, ps, bias_ap,
                                                       lk[:, 0:n], OP.add, OP.max)
                return f

            G1 = ST("G1")
            g1taps = [(0, 128, [(lhsT1(k), X, 0, 120, k - 8) for k in range(3)])]
            conv_pass(G1, (14, 526), g1taps,
                      act_evac(AF.Prelu, 0.0, 0.02), cast=False, bs=[0])
            conv_pass(G1, (526, Tc + 34), g1taps,
                      act_evac(AF.Prelu, 0.0, 0.02), cast=False, bs=[0],
                      runt_evac=dve_leaky(0.02), evac_dve=dve_leaky(0.02))
            conv_pass(G1, (14, Tc + 34), g1taps,
                      act_evac(AF.Prelu, 0.0, 0.02), cast=False, bs=[1],
                      runt_evac=dve_leaky(0.02), evac_dve=dve_leaky(0.02))
            E1 = ST("E1")
            conv_pass(E1, (13, Tc + 21),
                      [(0, 128, [(lhsT1(k), X, 0, 120, k - 8) for k in (1, 2)])],
                      act_evac(AF.Prelu, 0.0, 0.02), cast=False,
                      runt_evac=dve_leaky(0.02), evac_dve=dve_leaky(0.02))
            G2 = ST("G2")
            conv_pass(G2, (17, Tc + 33),
                      [(0, 128, [(lhsT(3 + k), G1, 0, 128, 2 * (k - 1)) for k in range(3)])],
                      act_evac(AF.Prelu, bias(0), 0.02),
                      runt_evac=dve_leaky(0.02, bias(0)),
                      evac_dve=dve_leaky(0.02, bias(0)))
            E2 = ST("E2")
            conv_pass(E2, (13, Tc + 21),
                      [(0, 128, [(lhsT(4), E1, 0, 128, 0), (lhsT(5), G1, 0, 128, 2)])],
                      act_evac(AF.Prelu, bias(0), 0.02),
                      runt_evac=dve_leaky(0.02, bias(0)),
                      evac_dve=dve_leaky(0.02, bias(0)))
            G3 = [ST("G3a"), ST("G3b")]
            for p in range(2):
                sl = 6 if p == 0 else 17
                conv_pass(G3[p], (21, Tc + 29),
                          [(0, 128, [(lhsT(sl + k), G2, 0, 128, 4 * (k - 1))
                                     for k in range(3)])],
                          act_evac(AF.Prelu, bias(1), 0.2),
                          runt_evac=dve_leaky(0.2, bias(1)),
                          evac_dve=dve_leaky(0.2, bias(1)))
            E3 = [ST("E3a"), ST("E3b")]
            for p in range(2):
                sl = 6 if p == 0 else 17
                conv_pass(E3[p], (13, Tc + 21),
                          [(0, 128, [(lhsT(sl + 1), E2, 0, 128, 0),
                                     (lhsT(sl + 2), G2, 0, 128, 4)])],
                          act_evac(AF.Prelu, bias(1), 0.2),
                          runt_evac=dve_leaky(0.2, bias(1)),
                          evac_dve=dve_leaky(0.2, bias(1)))
            H = [ST("Ha"), ST("Hb")]
            for p in range(2):
                conv_pass(H[p], (13, Tc + 21),
                          [(0, 128, [(lhsT(9), E3[p], 0, 128, 0),
                                     (lhsT(10), G3[p], 0, 128, 8)])],
                          act_evac(AF.Prelu, bias(2), 0.2),
                          runt_evac=dve_leaky(0.2, bias(2)),
                          evac_dve=dve_leaky(0.2, bias(2)))
            H1 = [ST("H1" + str(cidx)) for cidx in range(4)]
            for cidx in range(4):
                p, half = cidx // 2, cidx % 2
                conv_pass(H1[cidx], (13, Tc + 21),
                          [(0, 128, [(lhsT(11 if half == 0 else 20), H[p],
                                      0, 128, 0)])],
                          act_evac(AF.Prelu, bias(3), 0.02),
                          runt_evac=dve_leaky(0.02, bias(3)),
                          evac_dve=dve_leaky(0.02, bias(3)))
            A2 = [ST("A2a"), ST("A2b")]
            FFC = ST("FFC")
            S1 = ST("S1")
            Y = ypool.tile([64, BPC * Tc], BF, tag="y")
            for b in range(BPC):
                for p in range(2):
                    conv_pass(A2[p], (13, Tc + 21),
                              [(0, 128, [(lhsT(12), H1[2 * p], 0, 128, 0),
                                         (lhsT(15), H1[2 * p + 1], 0, 128, 0)])],
                              act_evac(AF.Prelu, bias(4), 0.02), bs=[b],
                              runt_evac=dve_leaky(0.02, bias(4)),
                              evac_dve=dve_leaky(0.02, bias(4)))
                conv_pass(FFC, (13, Tc + 21),
                          [(0, 128, [(lhsT(13), A2[0], 0, 128, 0),
                                     (lhsT(16), A2[1], 0, 128, 0)])],
                          act_evac(AF.Tanh, bias(5), 0.0, rnd=False), bs=[b])
                nc.gpsimd.memset(FFC[0:32, b * W + 13: b * W + 16], 0.0)
                nc.gpsimd.memset(FFC[96:128, b * W + Tc + 16: b * W + Tc + 21], 0.0)
                # 2-level window-sum tree for this batch on DVE (the last
                # doubling is folded into the final fc as a second matmul
                # tap); emitted here so it overlaps the next batch's A2/FFC
                o = b * W
                for (lo, hi) in _split(13, Tc + 19, PS_GROUP):
                    nc.vector.tensor_tensor(S1[:, o + lo: o + hi],
                                            FFC[:, o + lo: o + hi],
                                            FFC[:, o + lo + 1: o + hi + 1], OP.add)
                for (lo, hi) in _split(13, Tc + 17, PS_GROUP):
                    nc.vector.tensor_tensor(FFC[:, o + lo: o + hi],
                                            S1[:, o + lo: o + hi],
                                            S1[:, o + lo + 2: o + hi + 2], OP.add)

            for b in range(BPC):
                for (glo, ghi) in _split(16, Tc + 16, PS_GROUP):
                    ps = ppool.tile([128, 2 * PS_GROUP], F32, tag="ps",
                                    name="ps")
                    for (lo, hi) in _split(glo, ghi, 512):
                        n, off = hi - lo, lo - glo
                        nc.tensor.matmul(ps[0:128, off:off + n], lhsT(14),
                                         FFC[:, b * W + lo - 3: b * W + hi - 3],
                                         start=True, stop=False)
                        nc.tensor.matmul(ps[0:128, off:off + n], lhsT(14),
                                         FFC[:, b * W + lo + 1: b * W + hi + 1],
                                         start=False, stop=True)
                    nc.vector.tensor_scalar(
                        Y[:, b * Tc + glo - 16: b * Tc + ghi - 16],
                        ps[0:64, 0:ghi - glo], 0.0, 1.0, OP.max, OP.min)
                    # drain this batch-half right away: one DMA covers all 4
                    # time-chunk partition groups via a 3D dram AP
                    nc.sync.dma_start(
                        out=o_d[b, :, :, glo - 16: ghi - 16],
                        in_=Y[0:64, b * Tc + glo - 16: b * Tc + ghi - 16])
    nc.finalize()
    return nc




def _get_program(reps=1):
    global _PROG
    if _PROG is None:
        _PROG = {}
    if reps not in _PROG:
        _PROG[reps] = _build_program(reps)
    return _PROG[reps]


def _prep_inputs(inputs):
    x = np.asarray(inputs['speech_features'], np.float32)
    xa = np.zeros((B, C + 1, T + 22), np.float32)
    xa[:, :C, 10:10 + T] = x
    xa[:, C, :] = 1.0
    xc = np.empty((B, 4, C + 1, Tc + 22), np.float32)
    for c in range(4):
        xc[:, c] = xa[:, :, c * Tc: c * Tc + Tc + 22]
    wp = _pack_weights({k: np.asarray(v, np.float32) for k, v in inputs.items()
                        if k != 'speech_features'})
    wp1 = np.ascontiguousarray(wp[:120, 0:384]).astype(BF16)
    wp = wp.astype(BF16)
    xcb = xc.astype(BF16)
    return [{"x": xcb[i * BPC:(i + 1) * BPC], "wpack": wp, "wpack1": wp1}
            for i in range(NCORES)]


def kernel(**inputs):
    from concourse.bass_utils import run_bass_kernel_spmd

    in_maps = _prep_inputs(inputs)
    nc = _get_program()
    res = run_bass_kernel_spmd(nc, in_maps, core_ids=list(range(NCORES)))
    # out is [BPC, 4, 16, Tc]: c4 time-chunks x classes x chunk-time
    outs = [r["out"].transpose(0, 1, 3, 2).reshape(BPC, T, 16)
            for r in res.results]
    return np.ascontiguousarray(np.concatenate(outs, axis=0).astype(np.float32))



# revision 15
# speedup vs baseline: 1.3496x; 1.3496x over previous
"""Trainium2 Bass kernel for nn_CNNGenerator (frame CNN + FC + window-sum + FC).

Key algebraic facts exploited (validated vs the reference):
  * softmax over a size-1 axis == 1.0, so the whole attention_conv stack is
    dead code; the bmm reduces to an 8-wide sliding-window sum of ffc.
  * The per-window stride-2 conv stack collapses into global conv streams:
    an "interior" stream g{1,2,3} and a "left-edge" stream e{1,2,3} per
    layer, plus a 2-tap combine (z).  Per t:
      g1[s] = b1 + sum_k W1k x[s+k-8]          e1[t] = b1 + W11 x[t-7] + W12 x[t-6]
      g2[s] = b2 + V0 G1[s-2] + V1 G1[s] + V2 G1[s+2]
      e2[t] = b2 + V1 E1[t] + V2 G1[t+2]
      g3[s] = b3 + U0 G2[s-4] + U1 G2[s] + U2 G2[s+4]
      e3[t] = b3 + U1 E2[t] + U2 G2[t+4]
      z[t]  = b4 + T1 E3[t] + T2 G3[t+8]
    (capitals = leaky-activated streams), then fc1/fc2/fc3+tanh,
    ws[t] = sum_{d=-3..4} ffc[t+d], out = clip(fcw @ ws, 0, 1).

Sharding: pure data parallel, 2 batch elements per core on 8 cores.
On-chip layout: time axis split in 4 chunks of 2048; 32-channel streams pack
4 chunks x 32ch on the 128 partitions, 64-channel streams pack 2 chunks x 64ch
(two tiles).  Weights are host-packed into block-diagonal lhsT matrices.
This revision adds: input pre-chunked per time-chunk on the host (contiguous
HBM blocks), input/weight DMAs split across the SP and Activation hardware
DGE queues so compute starts earlier, weight pack loaded in two stages
(layer-1 slots first), window-sum tree chunked to 1024-col pieces so the
final fc matmuls pipeline with it, and output DMA split in column halves
for earlier drain.
"""
import sys

if '/opt/trn_rl_repo' not in sys.path:
    sys.path.insert(0, '/opt/trn_rl_repo')

import numpy as np
import ml_dtypes

BF16 = ml_dtypes.bfloat16

B, C, T = 16, 29, 8192
NCORES = 8
BPC = B // NCORES          # batch per core
Tc = T // 4                # time chunk
HL = 16
W = Tc + 40                # per-batch stream tile width
W2 = BPC * W
NSLOT = 21                 # 128-col lhsT slots in the weight pack
NWCOL = NSLOT * 128 + 8    # + bias columns

_PROG = {}
PS_GROUP = 1024
PS_BUFS = 4


def _blockdiag(blocks):
    k = sum(b.shape[0] for b in blocks)
    m = sum(b.shape[1] for b in blocks)
    out = np.zeros((k, m), np.float32)
    i = j = 0
    for b in blocks:
        out[i:i + b.shape[0], j:j + b.shape[1]] = b
        i += b.shape[0]
        j += b.shape[1]
    return out


def _pack_weights(inp):
    wp = np.zeros((128, NWCOL), np.float32)

    def put(slot, mat):
        wp[:mat.shape[0], slot * 128: slot * 128 + mat.shape[1]] = mat

    w1 = inp['w1'].astype(np.float32)  # [32, 29, 3]
    b1 = inp['b1'].astype(np.float32)
    for k in range(3):
        blk = np.zeros((30, 32), np.float32)
        blk[:29] = w1[:, :, k].T
        if k == 1:
            blk[29] = b1
        put(k, _blockdiag([blk] * 4))
    w2 = inp['w2'].astype(np.float32)
    for k in range(3):  # slots 3-5
        put(3 + k, _blockdiag([w2[:, :, k].T.astype(np.float32)] * 4))
    w3 = inp['w3'].astype(np.float32)
    for k in range(3):  # slots 6-8 (chunk-pair 0) / 17-19 (pair 1): zero-padded
        # to 128 contraction rows so the matmul runs in full 128x128 mode
        blk = _blockdiag([w3[:, :, k].T.astype(np.float32)] * 2)
        zb = np.zeros_like(blk)
        put(6 + k, np.concatenate([blk, zb], axis=0))
        put(17 + k, np.concatenate([zb, blk], axis=0))
    w4 = inp['w4'].astype(np.float32)
    for j in (1, 2):    # slots 9-10
        put(9 + j - 1, _blockdiag([w4[:, :, j].T.astype(np.float32)] * 2))
    fw1t = inp['fw1'].T.astype(np.float32)
    zf = np.zeros_like(fw1t)
    put(11, np.concatenate([fw1t, zf], axis=0))
    put(20, np.concatenate([zf, fw1t], axis=0))
    fw2t = inp['fw2'].T.astype(np.float32)
    z64 = np.zeros_like(fw2t)
    put(12, np.concatenate([fw2t, z64], axis=1))
    put(15, np.concatenate([z64, fw2t], axis=1))
    fw3t = _blockdiag([inp['fw3'].T.astype(np.float32)] * 2)
    z64b = np.zeros_like(fw3t)
    put(13, np.concatenate([fw3t, z64b], axis=1))
    put(16, np.concatenate([z64b, fw3t], axis=1))
    fcb = _blockdiag([inp['fcw'].T.astype(np.float32)] * 4)  # [128, 64]
    put(14, np.concatenate([fcb, np.zeros_like(fcb)], axis=1))  # M=128
    bc = NSLOT * 128
    wp[:, bc + 0] = np.tile(inp['b2'], 4)
    wp[:, bc + 1] = np.tile(inp['b3'], 2)
    wp[:, bc + 2] = np.tile(inp['b4'], 2)
    wp[:, bc + 3] = inp['fb1']
    wp[:, bc + 4] = np.tile(inp['fb2'], 2)
    wp[:, bc + 5] = np.tile(inp['fb3'], 4)
    return wp


def _split(lo, hi, step=512):
    return [(a, min(a + step, hi)) for a in range(lo, hi, step)]



def _build_program(reps=1):
    import concourse.bacc as bacc
    import concourse.mybir as mybir
    import concourse.tile as tile

    F32 = mybir.dt.float32
    F32R = mybir.dt.float32r
    BF = mybir.dt.bfloat16
    AF = mybir.ActivationFunctionType
    OP = mybir.AluOpType

    nc = bacc.Bacc("TRN2", target_bir_lowering=False, debug=False)
    x_d = nc.dram_tensor("x", [BPC, 4, C + 1, Tc + 22], BF, kind="ExternalInput").ap()
    w1_d = nc.dram_tensor("wpack1", [120, 384], BF, kind="ExternalInput").ap()
    w_d = nc.dram_tensor("wpack", [128, NWCOL], BF, kind="ExternalInput").ap()
    o_d = nc.dram_tensor("out", [BPC, 4, 16, Tc], BF, kind="ExternalOutput").ap()

    with tile.TileContext(nc) as tc:
        with tc.tile_pool(name="wp", bufs=1) as wpool, \
             tc.tile_pool(name="xp", bufs=1) as xpool, \
             tc.tile_pool(name="yp", bufs=1) as ypool, \
             tc.tile_pool(name="st", bufs=10) as spool, \
             tc.tile_pool(name="lk", bufs=2) as lkpool, \
             tc.tile_pool(name="ps", bufs=PS_BUFS, space="PSUM") as ppool:

            wsb = wpool.tile([128, NWCOL], BF, tag="w")
            wsb1 = wpool.tile([120, 384], BF, tag="w1")
            bfs = wpool.tile([128, 8], F32, tag="bf")
            nc.scalar.dma_start(out=wsb1[:], in_=w1_d[:])

            def lhsT(slot, k=128, m=128, base=0):
                return wsb[base:base + k, slot * 128: slot * 128 + m]

            def lhsT1(slot):
                return wsb1[0:120, slot * 128: slot * 128 + 128]

            def bias(i):
                return bfs[:, i:i + 1]

            X = xpool.tile([120, W2], BF, tag="x")
            nc.sync.dma_start(out=X[0:120, 0 * W + 6: 0 * W + Tc + 28],
                              in_=x_d[0])
            nc.sync.dma_start(out=X[0:120, 1 * W + 6: 1 * W + Tc + 28],
                              in_=x_d[1])
            nc.sync.dma_start(out=wsb[:, NSLOT * 128:],
                              in_=w_d[:, NSLOT * 128:])
            nc.scalar.activation(bfs[:, 0:6],
                                 wsb[:, NSLOT * 128: NSLOT * 128 + 6],
                                 mybir.ActivationFunctionType.Copy)
            nc.sync.dma_start(out=wsb[:, 0: NSLOT * 128],
                              in_=w_d[:, 0: NSLOT * 128])

            # PE warm-up: ~3.4us of dummy matmuls during the input DMAs flips
            # the HAM clock gate to 8/8 before real work arrives.
            wmt = xpool.tile([128, 512], F32, tag="wm")
            nc.gpsimd.memset(wmt[:], 0.0)
            psw = ppool.tile([128, PS_GROUP], F32, tag="ps", name="warm")
            for _w in range(8):
                nc.tensor.matmul(psw[:, 0:512], wmt[0:128, 0:128].bitcast(F32R),
                                 wmt[:, 0:512].bitcast(F32R),
                                 start=True, stop=True)

            ST = lambda nm: spool.tile([128, W2], BF, tag="st", name=nm)  # noqa: E731

            # Round-robin: every DVE_SHAREth full conv group evacuates on DVE
            # instead of Act, spreading evac load uniformly across the
            # timeline (whole-stream assignment phase-imbalances the engines).
            evcnt = [0]
            DVE_SHARE = 4

            def conv_pass(out_tile, rng, groups, evac, bs=None, cast=True,
                          runt_evac=None, evac_dve=None):
                for b in (range(BPC) if bs is None else bs):
                    for (glo, ghi) in _split(rng[0], rng[1], PS_GROUP):
                        gn = ghi - glo
                        if gn <= 16:
                            ev = evac
                        elif evac_dve is not None:
                            ev = evac_dve if evcnt[0] % DVE_SHARE == (DVE_SHARE - 1) \
                                else evac
                            evcnt[0] += 1
                        else:
                            ev = evac
                        ps = ppool.tile([128, PS_GROUP], F32, tag="ps", name="ps")
                        for (p0, p1, taps) in groups:
                            for (lo, hi) in _split(glo, ghi, 512):
                                n, off = hi - lo, lo - glo
                                for i, (lw, rt, rp0, rp1, d) in enumerate(taps):
                                    tp = (lw.base_partition(), p0) if p0 else None
                                    r = rt[rp0:rp1, b * W + lo + d: b * W + hi + d]
                                    nc.tensor.matmul(
                                        ps[p0:p1, off:off + n], lw, r,
                                        start=(i == 0), stop=(i == len(taps) - 1),
                                        tile_position=tp)
                        ev(ps[:, 0:gn], out_tile[:, b * W + glo: b * W + ghi])

            def act_evac(func, bias_ap, alpha, rnd=True):
                def f(ps, ot):
                    nc.scalar.activation(ot, ps, func, bias=bias_ap, scale=1.0,
                                         alpha=alpha)
                return f

            def dve_leaky(alpha, bias_ap=None):
                def f(ps, ot):
                    lk = lkpool.tile([128, 2 * PS_GROUP], BF, tag="lk", name="lk")
                    n = ps.free_size()
                    if bias_ap is None:
                        nc.vector.tensor_scalar(lk[:, 0:n], ps, alpha, None,
                                                OP.mult)
                        nc.vector.tensor_tensor(ot, ps, lk[:, 0:n], OP.max)
                    else:
                        nc.vector.tensor_scalar(lk[:, 0:n], ps, bias_ap, alpha,
                                                OP.add, OP.mult)
                        nc.vector.scalar_tensor_tensor(ot, ps, bias_ap,
                                                       lk[:, 0:n], OP.add, OP.max)
                return f

            def pool_leaky(alpha, bias_ap=None):
                def f(ps, ot):
                    lk = lkpool.tile([128, PS_GROUP], BF, tag="plk", name="plk")
                    n = ps.shape[-1]
                    if bias_ap is None:
                        nc.gpsimd.tensor_scalar(lk[:, 0:n], ps, alpha, None,
                                                OP.mult)
                        nc.gpsimd.tensor_tensor(ot, ps, lk[:, 0:n], OP.max)
                    else:
                        nc.gpsimd.tensor_scalar(lk[:, 0:n], ps, bias_ap, alpha,
                                                OP.add, OP.mult)
                        nc.gpsimd.scalar_tensor_tensor(ot, ps, bias_ap,
                                                       lk[:, 0:n], OP.add, OP.max)
                return f

            G1 = ST("G1")
            g1taps = [(0, 128, [(lhsT1(k), X, 0, 120, k - 8) for k in range(3)])]
            conv_pass(G1, (14, 526), g1taps,
                      act_evac(AF.Prelu, 0.0, 0.02), cast=False, bs=[0])
            conv_pass(G1, (526, Tc + 34), g1taps,
                      act_evac(AF.Prelu, 0.0, 0.02), cast=False, bs=[0],
                      runt_evac=dve_leaky(0.02), evac_dve=dve_leaky(0.02))
            conv_pass(G1, (14, Tc + 34), g1taps,
                      act_evac(AF.Prelu, 0.0, 0.02), cast=False, bs=[1],
                      runt_evac=dve_leaky(0.02), evac_dve=dve_leaky(0.02))
            E1 = ST("E1")
            conv_pass(E1, (13, Tc + 21),
                      [(0, 128, [(lhsT1(k), X, 0, 120, k - 8) for k in (1, 2)])],
                      act_evac(AF.Prelu, 0.0, 0.02), cast=False,
                      runt_evac=dve_leaky(0.02), evac_dve=dve_leaky(0.02))
            G2 = ST("G2")
            conv_pass(G2, (17, Tc + 33),
                      [(0, 128, [(lhsT(3 + k), G1, 0, 128, 2 * (k - 1)) for k in range(3)])],
                      act_evac(AF.Prelu, bias(0), 0.02),
                      runt_evac=dve_leaky(0.02, bias(0)),
                      evac_dve=dve_leaky(0.02, bias(0)))
            E2 = ST("E2")
            conv_pass(E2, (13, Tc + 21),
                      [(0, 128, [(lhsT(4), E1, 0, 128, 0), (lhsT(5), G1, 0, 128, 2)])],
                      act_evac(AF.Prelu, bias(0), 0.02),
                      runt_evac=dve_leaky(0.02, bias(0)),
                      evac_dve=dve_leaky(0.02, bias(0)))
            G3 = [ST("G3a"), ST("G3b")]
            for p in range(2):
                sl = 6 if p == 0 else 17
                conv_pass(G3[p], (21, Tc + 29),
                          [(0, 128, [(lhsT(sl + k), G2, 0, 128, 4 * (k - 1))
                                     for k in range(3)])],
                          act_evac(AF.Prelu, bias(1), 0.2),
                          runt_evac=dve_leaky(0.2, bias(1)),
                          evac_dve=dve_leaky(0.2, bias(1)))
            E3 = [ST("E3a"), ST("E3b")]
            for p in range(2):
                sl = 6 if p == 0 else 17
                conv_pass(E3[p], (13, Tc + 21),
                          [(0, 128, [(lhsT(sl + 1), E2, 0, 128, 0),
                                     (lhsT(sl + 2), G2, 0, 128, 4)])],
                          act_evac(AF.Prelu, bias(1), 0.2),
                          runt_evac=dve_leaky(0.2, bias(1)),
                          evac_dve=dve_leaky(0.2, bias(1)))
            H = [ST("Ha"), ST("Hb")]
            for p in range(2):
                conv_pass(H[p], (13, Tc + 21),
                          [(0, 128, [(lhsT(9), E3[p], 0, 128, 0),
                                     (lhsT(10), G3[p], 0, 128, 8)])],
                          act_evac(AF.Prelu, bias(2), 0.2),
                          runt_evac=dve_leaky(0.2, bias(2)),
                          evac_dve=dve_leaky(0.2, bias(2)))
            H1 = [ST("H1" + str(cidx)) for cidx in range(4)]
            for cidx in range(4):
                p, half = cidx // 2, cidx % 2
                conv_pass(H1[cidx], (13, Tc + 21),
                          [(0, 128, [(lhsT(11 if half == 0 else 20), H[p],
                                      0, 128, 0)])],
                          act_evac(AF.Prelu, bias(3), 0.02),
                          runt_evac=dve_leaky(0.02, bias(3)),
                          evac_dve=dve_leaky(0.02, bias(3)))
            A2 = [ST("A2a"), ST("A2b")]
            FFC = ST("FFC")
            S1 = ST("S1")
            Y = ypool.tile([64, BPC * Tc], BF, tag="y")
            for b in range(BPC):
                for p in range(2):
                    conv_pass(A2[p], (13, Tc + 21),
                              [(0, 128, [(lhsT(12), H1[2 * p], 0, 128, 0),
                                         (lhsT(15), H1[2 * p + 1], 0, 128, 0)])],
                              act_evac(AF.Prelu, bias(4), 0.02), bs=[b],
                              runt_evac=dve_leaky(0.02, bias(4)),
                              evac_dve=dve_leaky(0.02, bias(4)))
                conv_pass(FFC, (13, Tc + 21),
                          [(0, 128, [(lhsT(13), A2[0], 0, 128, 0),
                                     (lhsT(16), A2[1], 0, 128, 0)])],
                          act_evac(AF.Tanh, bias(5), 0.0, rnd=False), bs=[b])
                nc.gpsimd.memset(FFC[0:32, b * W + 13: b * W + 16], 0.0)
                nc.gpsimd.memset(FFC[96:128, b * W + Tc + 16: b * W + Tc + 21], 0.0)
                # 2-level window-sum tree for this batch on DVE (the last
                # doubling is folded into the final fc as a second matmul
                # tap); emitted here so it overlaps the next batch's A2/FFC
                o = b * W
                for (lo, hi) in _split(13, Tc + 19, PS_GROUP):
                    nc.vector.tensor_tensor(S1[:, o + lo: o + hi],
                                            FFC[:, o + lo: o + hi],
                                            FFC[:, o + lo + 1: o + hi + 1], OP.add)
                for (lo, hi) in _split(13, Tc + 17, PS_GROUP):
                    nc.vector.tensor_tensor(FFC[:, o + lo: o + hi],
                                            S1[:, o + lo: o + hi],
                                            S1[:, o + lo + 2: o + hi + 2], OP.add)

            for b in range(BPC):
                for (glo, ghi) in _split(16, Tc + 16, PS_GROUP):
                    ps = ppool.tile([128, PS_GROUP], F32, tag="ps", name="ps")
                    for (lo, hi) in _split(glo, ghi, 512):
                        n, off = hi - lo, lo - glo
                        nc.tensor.matmul(ps[0:128, off:off + n], lhsT(14),
                                         FFC[:, b * W + lo - 3: b * W + hi - 3],
                                         start=True, stop=False)
                        nc.tensor.matmul(ps[0:128, off:off + n], lhsT(14),
                                         FFC[:, b * W + lo + 1: b * W + hi + 1],
                                         start=False, stop=True)
                    nc.vector.tensor_scalar(
                        Y[:, b * Tc + glo - 16: b * Tc + ghi - 16],
                        ps[0:64, 0:ghi - glo], 0.0, 1.0, OP.max, OP.min)
                    # drain this batch-half right away: one DMA covers all 4
                    # time-chunk partition groups via a 3D dram AP
                    nc.sync.dma_start(
                        out=o_d[b, :, :, glo - 16: ghi - 16],
                        in_=Y[0:64, b * Tc + glo - 16: b * Tc + ghi - 16])
    nc.finalize()
    return nc




def _get_program(reps=1):
    global _PROG
    if _PROG is None:
        _PROG = {}
    if reps not in _PROG:
        _PROG[reps] = _build_program(reps)
    return _PROG[reps]


def _prep_inputs(inputs):
    x = np.asarray(inputs['speech_features'], np.float32)
    xa = np.zeros((B, C + 1, T + 22), np.float32)
    xa[:, :C, 10:10 + T] = x
    xa[:, C, :] = 1.0
    xc = np.empty((B, 4, C + 1, Tc + 22), np.float32)
    for c in range(4):
        xc[:, c] = xa[:, :, c * Tc: c * Tc + Tc + 22]
    wp = _pack_weights({k: np.asarray(v, np.float32) for k, v in inputs.items()
                        if k != 'speech_features'})
    wp1 = np.ascontiguousarray(wp[:120, 0:384]).astype(BF16)
    wp = wp.astype(BF16)
    xcb = xc.astype(BF16)
    return [{"x": xcb[i * BPC:(i + 1) * BPC], "wpack": wp, "wpack1": wp1}
            for i in range(NCORES)]


def kernel(**inputs):
    from concourse.bass_utils import run_bass_kernel_spmd

    in_maps = _prep_inputs(inputs)
    nc = _get_program()
    res = run_bass_kernel_spmd(nc, in_maps, core_ids=list(range(NCORES)))
    # out is [BPC, 4, 16, Tc]: c4 time-chunks x classes x chunk-time
    outs = [r["out"].transpose(0, 1, 3, 2).reshape(BPC, T, 16)
            for r in res.results]
    return np.ascontiguousarray(np.concatenate(outs, axis=0).astype(np.float32))

